# revision 1
# baseline (speedup 1.0000x reference)
"""Trainium2 Bass kernel for nn_MinimumSpanningTree.

Contract: kernel(**inputs) takes the FULL inputs (guide_in [8, 64, 256, 256]
f32) and returns the FULL output (tree [8, 65535, 2] int32).

Strategy (data-parallel over batch, one image per NeuronCore):
  - Device (Bass, 8 cores SPMD): the memory-bound edge-weight build.
    For each image, squared-L2-over-channels distances for the 130560 grid
    edges, with the channel reduction done in the same sequential order as
    the reference (verified bitwise-identical): DVE subtract -> ACT square
    -> PE transpose (pixel-major) -> DVE grouped tensor_reduce.
  - Boruvka MST per image (exactly the reference algorithm) + output
    assembly.

Self-contained: shapes/sharding hardcoded.
"""
import numpy as np

B, C, H, W = 8, 64, 256, 256
V = H * W
E_ROW = (H - 1) * W
E_COL = H * (W - 1)
E = E_ROW + E_COL
N_ROUNDS = 16

_compiled = None


def _build_program():
    """Build + compile the SPMD bass program (one image per core)."""
    import concourse.bacc as bacc
    import concourse.mybir as mybir
    from concourse import tile
    from concourse.masks import make_identity

    F32 = mybir.dt.float32
    AL = mybir.AluOpType
    ACT = mybir.ActivationFunctionType

    PIX = V              # 65536 pixels per image
    PAD = 260
    CHUNK = 2048         # pixels per chunk
    NPC = 16             # pair-chunks: pc pairs chunk pc (A) with pc+16 (B)

    nc = bacc.Bacc('TRN2', target_bir_lowering=False, debug=False, num_devices=8)
    d_fm = nc.dram_tensor("fm", [C, PIX + PAD], F32, kind="ExternalInput")
    # packed layout: col pc*32 + 2t + b holds pixel (pc + 16*b)*2048 + 128*t + p
    o_dr = nc.dram_tensor("drow", [128, 512], F32, kind="ExternalOutput")
    o_dc = nc.dram_tensor("dcol", [128, 512], F32, kind="ExternalOutput")

    with tile.TileContext(nc) as tc:
        with tc.tile_pool(name="pool", bufs=4) as pool, \
             tc.tile_pool(name="acc", bufs=1) as accp, \
             tc.tile_pool(name="cst", bufs=1) as cstp, \
             tc.tile_pool(name="ps", bufs=2, space="PSUM") as psum:
            ident = cstp.tile([128, 128], F32)
            make_identity(nc, ident[:])
            dRT = accp.tile([128, 512], F32)
            dCT = accp.tile([128, 512], F32)

            for pc in range(NPC):
                t = pool.tile([128, CHUNK + 257], F32, tag="in")
                a0 = pc * CHUNK
                b0 = (pc + 16) * CHUNK
                nc.sync.dma_start(t[0:64, :], d_fm[:, a0: a0 + CHUNK + 257])
                nc.sync.dma_start(t[64:128, :], d_fm[:, b0: b0 + CHUNK + 257])

                dr = pool.tile([128, CHUNK], F32, tag="dr")
                dc = pool.tile([128, CHUNK], F32, tag="dc")
                # split subtracts DVE/GPSIMD to balance engine busy time
                e1 = nc.vector if pc % 3 == 2 else nc.gpsimd
                e2 = nc.vector if pc % 3 == 1 else nc.gpsimd
                e1.tensor_tensor(dr[:], t[:, 0:CHUNK], t[:, 256:CHUNK + 256], AL.subtract)
                e2.tensor_tensor(dc[:], t[:, 0:CHUNK], t[:, 1:CHUNK + 1], AL.subtract)

                sr = pool.tile([128, CHUNK], F32, tag="sr")
                sc = pool.tile([128, CHUNK], F32, tag="sc")
                nc.scalar.activation(sr[:], dr[:], ACT.Square)
                nc.scalar.activation(sc[:], dc[:], ACT.Square)

                # transpose to pixel-major (row = pixel, free = [chA 64ch | chB 64ch])
                for half in range(2):  # 1024 pixels -> 8 transposes -> one PSUM [128, 1024]
                    pr = psum.tile([128, 1024], F32, tag="pr")
                    pcm = psum.tile([128, 1024], F32, tag="pcm")
                    for q in range(8):
                        off = half * 1024 + q * 128
                        nc.tensor.transpose(pr[:, q * 128:(q + 1) * 128],
                                            sr[:, off:off + 128], ident[:])
                        nc.tensor.transpose(pcm[:, q * 128:(q + 1) * 128],
                                            sc[:, off:off + 128], ident[:])
                    colbase = pc * 32 + half * 16
                    nc.vector.tensor_reduce(
                        dRT[:, colbase:colbase + 16],
                        pr[:].rearrange("p (g k) -> p g k", k=64),
                        mybir.AxisListType.X, AL.add)
                    nc.vector.tensor_reduce(
                        dCT[:, colbase:colbase + 16],
                        pcm[:].rearrange("p (g k) -> p g k", k=64),
                        mybir.AxisListType.X, AL.add)

            nc.sync.dma_start(o_dr[:], dRT[:])
            nc.sync.dma_start(o_dc[:], dCT[:])

    nc.compile()
    return nc


def _get_program():
    global _compiled
    if _compiled is None:
        _compiled = _build_program()
    return _compiled


def _edge_weights_device(guide_in):
    """Run the bass program on 8 cores; returns (wr [B,255,256], wc [B,256,255])."""
    from concourse.bass_utils import run_bass_kernel_spmd

    nc = _get_program()
    pad = np.zeros((C, 260), np.float32)
    in_maps = []
    for b in range(B):
        fm = np.ascontiguousarray(guide_in[b].reshape(C, V))
        in_maps.append({"fm": np.concatenate([fm, pad], axis=1)})
    res = run_bass_kernel_spmd(nc, in_maps, list(range(8)))

    def decode(arr):
        # col pc*32 + half*8 + q*2 + b <-> pixel (pc+16b)*2048 + half*512 + q*128 + p
        a = np.asarray(arr).reshape(128, 16, 4, 4, 2)
        return a.transpose(4, 1, 2, 3, 0).reshape(-1)

    wr, wc = [], []
    for b in range(B):
        r = res.results[b]
        drow = decode(r["drow"])[:E_ROW]
        dcol = decode(r["dcol"]).reshape(H, W)[:, :W - 1]
        wr.append(drow.reshape(H - 1, W) + np.float32(1.0))
        wc.append(dcol + np.float32(1.0))
    return np.stack(wr), np.stack(wc)


def _build_index():
    raw = np.arange(V, dtype=np.int32).reshape(H, W)
    row_e = np.stack([raw[:-1, :], raw[1:, :]], axis=-1).reshape(-1, 2)
    col_e = np.stack([raw[:, :-1], raw[:, 1:]], axis=-1).reshape(-1, 2)
    return np.concatenate([row_e, col_e], axis=0)


def _scatter_min(target, keys, vals):
    """target[k] = min(target[k], min of vals where keys==k), fast path."""
    order = np.argsort(keys, kind="stable")
    ks = keys[order]
    vs = vals[order]
    starts = np.flatnonzero(np.r_[True, ks[1:] != ks[:-1]])
    mins = np.minimum.reduceat(vs, starts)
    target[ks[starts]] = np.minimum(target[ks[starts]], mins)


def _mst_boruvka(u, v, w):
    """Exact port of the reference Boruvka (per image)."""
    eidx = np.arange(E, dtype=np.int64)
    vidx = np.arange(V, dtype=np.int64)
    INF = np.float32(np.inf)
    BIGE = E
    comp = vidx.copy()
    sel = np.zeros(E, dtype=bool)
    for _ in range(N_ROUNDS):
        cu, cv = comp[u], comp[v]
        active = cu != cv
        if not active.any():
            break
        wa = np.where(active, w, INF)
        minw = np.full(V, INF, np.float32)
        _scatter_min(minw, cu, wa)
        _scatter_min(minw, cv, wa)
        cand_u = np.where(active & (wa == minw[cu]), eidx, BIGE)
        cand_v = np.where(active & (wa == minw[cv]), eidx, BIGE)
        best = np.full(V, BIGE, np.int64)
        _scatter_min(best, cu, cand_u)
        _scatter_min(best, cv, cand_v)
        has = best < BIGE
        be = np.clip(best, 0, E - 1)
        cu_b, cv_b = comp[u[be]], comp[v[be]]
        parent = np.where(has, np.where(cu_b == vidx, cv_b, cu_b), vidx)
        pp = parent[parent]
        parent = np.where((pp == vidx) & (vidx < parent), vidx, parent)
        for _ in range(N_ROUNDS):
            parent = parent[parent]
        comp = parent[comp]
        sel_idx = best[has]
        sel[sel_idx] = True
    return sel


def kernel(guide_in):
    guide_in = np.asarray(guide_in, dtype=np.float32)
    wr, wc = _edge_weights_device(guide_in)

    index = _build_index()
    u = index[:, 0].astype(np.int64)
    v = index[:, 1].astype(np.int64)
    trees = []
    for b in range(B):
        w = np.concatenate([wr[b].reshape(-1), wc[b].reshape(-1)]).astype(np.float32)
        sel = _mst_boruvka(u, v, w)
        eids = np.nonzero(sel)[0]
        if len(eids) != V - 1:  # pad/trim defensively (should be exactly V-1)
            eids = np.concatenate([eids, np.zeros(max(0, V - 1 - len(eids)), np.int64)])[:V - 1]
        trees.append(index[eids])
    return np.stack(trees).astype(np.int32)



# revision 3
# speedup vs baseline: 1.5693x; 1.5693x over previous
"""Trainium2 Bass kernel for nn_MinimumSpanningTree.

Contract: kernel(**inputs) takes the FULL inputs (guide_in [8, 64, 256, 256]
f32) and returns the FULL output (tree [8, 65535, 2] int32).

Strategy (data-parallel over batch, one image per NeuronCore):
  Device computes, per pixel p, three channel-reductions:
    S[p]  = sum_c fm[c,p]^2
    MR[p] = sum_c fm[c,p]*fm[c,p+256]   (row-neighbor cross term)
    MC[p] = sum_c fm[c,p]*fm[c,p+1]     (col-neighbor cross term)
  via: ACT square / DVE+GPSIMD mults (f32r outputs) and f32r matmul-by-ones
  channel reduction on the PE (shifted-ones lhsT accumulating into dense
  [64, 512] PSUM tiles).
  Host assembles edge weights with the exact identity
    ||a-b||^2 = S[a] + S[b] - 2*a.b
  and runs the reference Boruvka MST per image.

Self-contained: shapes/sharding hardcoded.
"""
import numpy as np

B, C, H, W = 8, 64, 256, 256
V = H * W
E_ROW = (H - 1) * W
E_COL = H * (W - 1)
E = E_ROW + E_COL
N_ROUNDS = 16

HALF = V // 2            # 32768 pixels per half
CHUNK = 2048             # pixels per pair-chunk (per half)
NPC = HALF // CHUNK      # 16 pair-chunks
HCOLS = HALF + CHUNK + 256  # 35072 columns per half in DRAM (incl. spill/pad)

_compiled = None


def _build_program():
    import concourse.bacc as bacc
    import concourse.mybir as mybir
    from concourse import tile

    F32 = mybir.dt.float32
    F32R = mybir.dt.float32r
    AL = mybir.AluOpType
    ACTF = mybir.ActivationFunctionType

    nc = bacc.Bacc('TRN2', target_bir_lowering=False, debug=False, num_devices=8)
    d_fm = nc.dram_tensor("fm2", [2, C, HCOLS], F32, kind="ExternalInput")
    d_out = nc.dram_tensor("red", [128, 1536], F32, kind="ExternalOutput")

    with tile.TileContext(nc) as tc:
        with tc.tile_pool(name="cst", bufs=1) as cstp, \
             tc.tile_pool(name="inp", bufs=3) as inpool, \
             tc.tile_pool(name="qt", bufs=2) as qpool, \
             tc.tile_pool(name="st", bufs=1) as stpool, \
             tc.tile_pool(name="ps", bufs=1, space="PSUM") as psum:
            # shifted-ones weights: ones at cols 62 (chA) / 63 (chB)
            ones_f = cstp.tile([128, 126], F32)
            nc.vector.memset(ones_f[:], 0.0)
            nc.vector.memset(ones_f[0:64, 62:63], 1.0)
            nc.vector.memset(ones_f[64:128, 63:64], 1.0)
            Tb = cstp.tile([128, 126], F32R)
            nc.scalar.copy(Tb[:], ones_f[:])

            P = [[psum.tile([64, 512], F32, name=f"P{q}{g}", tag=f"p{q}{g}") for g in range(2)]
                 for q in range(3)]
            stage = [stpool.tile([64, 1536], F32, name=f"stage{g}", tag=f"stg{g}") for g in range(2)]

            mult_i = 0
            for pc in range(NPC):
                t = inpool.tile([128, CHUNK + 256], F32, tag="in")
                nc.sync.dma_start(t[:], d_fm[:, :, CHUNK * pc: CHUNK * pc + CHUNK + 256])

                sq = qpool.tile([128, CHUNK], F32R, tag="sq")
                mr = qpool.tile([128, CHUNK], F32R, tag="mr")
                mc = qpool.tile([128, CHUNK], F32R, tag="mc")
                nc.scalar.activation(sq[:], t[:, 0:CHUNK], ACTF.Square)
                for dst, off in ((mr, 256), (mc, 1)):
                    eng = nc.gpsimd if mult_i % 3 == 0 else nc.vector
                    eng.tensor_tensor(dst[:], t[:, 0:CHUNK], t[:, off:CHUNK + off], AL.mult)
                    mult_i += 1

                g = pc // 8
                s0 = (pc % 8) * 4
                for q, qt in ((0, sq), (1, mr), (2, mc)):
                    for k in range(4):
                        s = s0 + k
                        nc.tensor.matmul(
                            P[q][g][:],
                            Tb[:, 62 - 2 * s:126 - 2 * s],
                            qt[:, 512 * k:512 * k + 512],
                            start=(s == 0), stop=(s == 31))

                if pc % 8 == 7:
                    for q in range(3):
                        nc.scalar.copy(stage[g][:, 512 * q:512 * q + 512], P[q][g][:])
                    nc.sync.dma_start(d_out[64 * g:64 * g + 64, :], stage[g][:])

    nc.compile()
    return nc


def _get_program():
    global _compiled
    if _compiled is None:
        _compiled = _build_program()
    return _compiled


def _device_reductions(guide_in):
    """Run the bass program on 8 cores; returns (S, MR, MC) each [B, V] f32."""
    from concourse.bass_utils import run_bass_kernel_spmd

    nc = _get_program()
    in_maps = []
    for b in range(B):
        fm = np.ascontiguousarray(guide_in[b].reshape(C, V))
        fm2 = np.zeros((2, C, HCOLS), np.float32)
        fm2[0] = fm[:, 0:HCOLS]
        fm2[1, :, :V - HALF] = fm[:, HALF:V]
        in_maps.append({"fm2": fm2})
    res = run_bass_kernel_spmd(nc, in_maps, list(range(8)))

    S = np.empty((B, V), np.float32)
    MR = np.empty((B, V), np.float32)
    MC = np.empty((B, V), np.float32)
    for b in range(B):
        out = np.asarray(res.results[b]["red"], np.float32)  # [128, 1536]
        # row 64g + 2s' + h, col 512q + j  ->  qty q, half h, px 16384g+512s'+j
        a = out.reshape(2, 32, 2, 3, 512)          # [g, s', h, q, j]
        a = a.transpose(3, 2, 0, 1, 4)             # [q, h, g, s', j]
        vals = a.reshape(3, 2 * V // 2)            # [q, h*px] -> [q, V]
        S[b], MR[b], MC[b] = vals[0], vals[1], vals[2]
    return S, MR, MC


def _weights_from_reductions(S, MR, MC):
    """w [E] from per-pixel reductions of one image (exact identity)."""
    w_row = S[:-256] + S[256:] - 2.0 * MR[:V - 256] + 1.0
    w_row = w_row[:E_ROW].astype(np.float32)
    w_colf = np.empty(V, np.float32)
    w_colf[:-1] = S[:-1] + S[1:] - 2.0 * MC[:V - 1] + 1.0
    w_colf[-1] = 0.0
    w_col = w_colf.reshape(H, W)[:, :W - 1].reshape(-1)
    return np.concatenate([w_row, w_col]).astype(np.float32)


def _build_index():
    raw = np.arange(V, dtype=np.int32).reshape(H, W)
    row_e = np.stack([raw[:-1, :], raw[1:, :]], axis=-1).reshape(-1, 2)
    col_e = np.stack([raw[:, :-1], raw[:, 1:]], axis=-1).reshape(-1, 2)
    return np.concatenate([row_e, col_e], axis=0)


def _scatter_min(target, keys, vals):
    order = np.argsort(keys, kind="stable")
    ks = keys[order]
    vs = vals[order]
    starts = np.flatnonzero(np.r_[True, ks[1:] != ks[:-1]])
    mins = np.minimum.reduceat(vs, starts)
    target[ks[starts]] = np.minimum(target[ks[starts]], mins)


def _mst_boruvka(u, v, w):
    """Exact port of the reference Boruvka (per image)."""
    eidx = np.arange(E, dtype=np.int64)
    vidx = np.arange(V, dtype=np.int64)
    INF = np.float32(np.inf)
    BIGE = E
    comp = vidx.copy()
    sel = np.zeros(E, dtype=bool)
    for _ in range(N_ROUNDS):
        cu, cv = comp[u], comp[v]
        active = cu != cv
        if not active.any():
            break
        wa = np.where(active, w, INF)
        minw = np.full(V, INF, np.float32)
        _scatter_min(minw, cu, wa)
        _scatter_min(minw, cv, wa)
        cand_u = np.where(active & (wa == minw[cu]), eidx, BIGE)
        cand_v = np.where(active & (wa == minw[cv]), eidx, BIGE)
        best = np.full(V, BIGE, np.int64)
        _scatter_min(best, cu, cand_u)
        _scatter_min(best, cv, cand_v)
        has = best < BIGE
        be = np.clip(best, 0, E - 1)
        cu_b, cv_b = comp[u[be]], comp[v[be]]
        parent = np.where(has, np.where(cu_b == vidx, cv_b, cu_b), vidx)
        pp = parent[parent]
        parent = np.where((pp == vidx) & (vidx < parent), vidx, parent)
        for _ in range(N_ROUNDS):
            parent = parent[parent]
        comp = parent[comp]
        sel[best[has]] = True
    return sel


def kernel(guide_in):
    guide_in = np.asarray(guide_in, dtype=np.float32)
    S, MR, MC = _device_reductions(guide_in)

    index = _build_index()
    u = index[:, 0].astype(np.int64)
    v = index[:, 1].astype(np.int64)
    trees = []
    for b in range(B):
        w = _weights_from_reductions(S[b], MR[b], MC[b])
        sel = _mst_boruvka(u, v, w)
        eids = np.nonzero(sel)[0]
        if len(eids) != V - 1:  # pad/trim defensively (should be exactly V-1)
            eids = np.concatenate([eids, np.zeros(max(0, V - 1 - len(eids)), np.int64)])[:V - 1]
        trees.append(index[eids])
    return np.stack(trees).astype(np.int32)


# revision 4
# speedup vs baseline: 1.6759x; 1.0679x over previous
"""Trainium2 Bass kernel for nn_MinimumSpanningTree.

Contract: kernel(**inputs) takes the FULL inputs (guide_in [8, 64, 256, 256]
f32) and returns the FULL output (tree [8, 65535, 2] int32).

Strategy (data-parallel over batch, one image per NeuronCore):
  Device computes, per pixel p, three channel-reductions:
    S[p]  = sum_c fm[c,p]^2
    MR[p] = sum_c fm[c,p]*fm[c,p+256]   (row-neighbor cross term)
    MC[p] = sum_c fm[c,p]*fm[c,p+1]     (col-neighbor cross term)
  via: ACT square / DVE+GPSIMD mults (f32r outputs) and f32r matmul-by-ones
  channel reduction on the PE (shifted-ones lhsT accumulating into dense
  [64, 512] PSUM tiles).
  Host assembles edge weights with the exact identity
    ||a-b||^2 = S[a] + S[b] - 2*a.b
  and runs the reference Boruvka MST per image.

Self-contained: shapes/sharding hardcoded.
"""
import numpy as np

B, C, H, W = 8, 64, 256, 256
V = H * W
E_ROW = (H - 1) * W
E_COL = H * (W - 1)
E = E_ROW + E_COL
N_ROUNDS = 16

HALF = V // 2            # 32768 pixels per half
CHUNK = 2048             # pixels per pair-chunk (per half)
NPC = HALF // CHUNK      # 16 pair-chunks
HCOLS = HALF + CHUNK + 256  # 35072 columns per half in DRAM (incl. spill/pad)

_compiled = None


def _build_program():
    import concourse.bacc as bacc
    import concourse.mybir as mybir
    from concourse import tile

    F32 = mybir.dt.float32
    F32R = mybir.dt.float32r
    AL = mybir.AluOpType
    ACTF = mybir.ActivationFunctionType

    nc = bacc.Bacc('TRN2', target_bir_lowering=False, debug=False, num_devices=8)
    d_fm = nc.dram_tensor("fm2", [2, C, HCOLS], F32, kind="ExternalInput")
    d_out = nc.dram_tensor("red", [128, 1536], F32, kind="ExternalOutput")

    with tile.TileContext(nc) as tc:
        with tc.tile_pool(name="cst", bufs=1) as cstp, \
             tc.tile_pool(name="inp", bufs=5) as inpool, \
             tc.tile_pool(name="qt", bufs=4) as qpool, \
             tc.tile_pool(name="st", bufs=1) as stpool, \
             tc.tile_pool(name="ps", bufs=1, space="PSUM") as psum:
            # shifted-ones weights: ones at cols 62 (chA) / 63 (chB)
            ones_f = cstp.tile([128, 126], F32)
            nc.vector.memset(ones_f[:], 0.0)
            nc.vector.memset(ones_f[0:64, 62:63], 1.0)
            nc.vector.memset(ones_f[64:128, 63:64], 1.0)
            Tb = cstp.tile([128, 126], F32R)
            nc.scalar.copy(Tb[:], ones_f[:])

            P = [[psum.tile([64, 512], F32, name=f"P{q}{g}", tag=f"p{q}{g}") for g in range(2)]
                 for q in range(3)]
            stage = [stpool.tile([64, 1536], F32, name=f"stage{g}", tag=f"stg{g}") for g in range(2)]

            mult_i = 0
            for pc in range(NPC):
                t = inpool.tile([128, CHUNK + 256], F32, tag="in")
                nc.sync.dma_start(t[:], d_fm[:, :, CHUNK * pc: CHUNK * pc + CHUNK + 256])

                sq = qpool.tile([128, CHUNK], F32R, tag="sq")
                mr = qpool.tile([128, CHUNK], F32R, tag="mr")
                mc = qpool.tile([128, CHUNK], F32R, tag="mc")
                nc.scalar.activation(sq[:], t[:, 0:CHUNK], ACTF.Square)
                for dst, off in ((mr, 256), (mc, 1)):
                    eng = nc.gpsimd if mult_i % 3 == 0 else nc.vector
                    eng.tensor_tensor(dst[:], t[:, 0:CHUNK], t[:, off:CHUNK + off], AL.mult)
                    mult_i += 1

                g = pc // 8
                s0 = (pc % 8) * 4
                for q, qt in ((0, sq), (1, mr), (2, mc)):
                    for k in range(4):
                        s = s0 + k
                        nc.tensor.matmul(
                            P[q][g][:],
                            Tb[:, 62 - 2 * s:126 - 2 * s],
                            qt[:, 512 * k:512 * k + 512],
                            start=(s == 0), stop=(s == 31))

                if pc % 8 == 7:
                    for q in range(3):
                        nc.scalar.copy(stage[g][:, 512 * q:512 * q + 512], P[q][g][:])
                    nc.sync.dma_start(d_out[64 * g:64 * g + 64, :], stage[g][:])

    nc.compile()
    return nc


def _get_program():
    global _compiled
    if _compiled is None:
        _compiled = _build_program()
    return _compiled


def _device_reductions(guide_in):
    """Run the bass program on 8 cores; returns (S, MR, MC) each [B, V] f32."""
    from concourse.bass_utils import run_bass_kernel_spmd

    nc = _get_program()
    in_maps = []
    for b in range(B):
        fm = np.ascontiguousarray(guide_in[b].reshape(C, V))
        fm2 = np.zeros((2, C, HCOLS), np.float32)
        fm2[0] = fm[:, 0:HCOLS]
        fm2[1, :, :V - HALF] = fm[:, HALF:V]
        in_maps.append({"fm2": fm2})
    res = run_bass_kernel_spmd(nc, in_maps, list(range(8)))

    S = np.empty((B, V), np.float32)
    MR = np.empty((B, V), np.float32)
    MC = np.empty((B, V), np.float32)
    for b in range(B):
        out = np.asarray(res.results[b]["red"], np.float32)  # [128, 1536]
        # row 64g + 2s' + h, col 512q + j  ->  qty q, half h, px 16384g+512s'+j
        a = out.reshape(2, 32, 2, 3, 512)          # [g, s', h, q, j]
        a = a.transpose(3, 2, 0, 1, 4)             # [q, h, g, s', j]
        vals = a.reshape(3, 2 * V // 2)            # [q, h*px] -> [q, V]
        S[b], MR[b], MC[b] = vals[0], vals[1], vals[2]
    return S, MR, MC


def _weights_from_reductions(S, MR, MC):
    """w [E] from per-pixel reductions of one image (exact identity)."""
    w_row = S[:-256] + S[256:] - 2.0 * MR[:V - 256] + 1.0
    w_row = w_row[:E_ROW].astype(np.float32)
    w_colf = np.empty(V, np.float32)
    w_colf[:-1] = S[:-1] + S[1:] - 2.0 * MC[:V - 1] + 1.0
    w_colf[-1] = 0.0
    w_col = w_colf.reshape(H, W)[:, :W - 1].reshape(-1)
    return np.concatenate([w_row, w_col]).astype(np.float32)


def _build_index():
    raw = np.arange(V, dtype=np.int32).reshape(H, W)
    row_e = np.stack([raw[:-1, :], raw[1:, :]], axis=-1).reshape(-1, 2)
    col_e = np.stack([raw[:, :-1], raw[:, 1:]], axis=-1).reshape(-1, 2)
    return np.concatenate([row_e, col_e], axis=0)


def _scatter_min(target, keys, vals):
    order = np.argsort(keys, kind="stable")
    ks = keys[order]
    vs = vals[order]
    starts = np.flatnonzero(np.r_[True, ks[1:] != ks[:-1]])
    mins = np.minimum.reduceat(vs, starts)
    target[ks[starts]] = np.minimum(target[ks[starts]], mins)


def _mst_boruvka(u, v, w):
    """Exact port of the reference Boruvka (per image)."""
    eidx = np.arange(E, dtype=np.int64)
    vidx = np.arange(V, dtype=np.int64)
    INF = np.float32(np.inf)
    BIGE = E
    comp = vidx.copy()
    sel = np.zeros(E, dtype=bool)
    for _ in range(N_ROUNDS):
        cu, cv = comp[u], comp[v]
        active = cu != cv
        if not active.any():
            break
        wa = np.where(active, w, INF)
        minw = np.full(V, INF, np.float32)
        _scatter_min(minw, cu, wa)
        _scatter_min(minw, cv, wa)
        cand_u = np.where(active & (wa == minw[cu]), eidx, BIGE)
        cand_v = np.where(active & (wa == minw[cv]), eidx, BIGE)
        best = np.full(V, BIGE, np.int64)
        _scatter_min(best, cu, cand_u)
        _scatter_min(best, cv, cand_v)
        has = best < BIGE
        be = np.clip(best, 0, E - 1)
        cu_b, cv_b = comp[u[be]], comp[v[be]]
        parent = np.where(has, np.where(cu_b == vidx, cv_b, cu_b), vidx)
        pp = parent[parent]
        parent = np.where((pp == vidx) & (vidx < parent), vidx, parent)
        for _ in range(N_ROUNDS):
            parent = parent[parent]
        comp = parent[comp]
        sel[best[has]] = True
    return sel


def kernel(guide_in):
    guide_in = np.asarray(guide_in, dtype=np.float32)
    S, MR, MC = _device_reductions(guide_in)

    index = _build_index()
    u = index[:, 0].astype(np.int64)
    v = index[:, 1].astype(np.int64)
    trees = []
    for b in range(B):
        w = _weights_from_reductions(S[b], MR[b], MC[b])
        sel = _mst_boruvka(u, v, w)
        eids = np.nonzero(sel)[0]
        if len(eids) != V - 1:  # pad/trim defensively (should be exactly V-1)
            eids = np.concatenate([eids, np.zeros(max(0, V - 1 - len(eids)), np.int64)])[:V - 1]
        trees.append(index[eids])
    return np.stack(trees).astype(np.int32)
